# revision 30
# baseline (speedup 1.0000x reference)
"""Dense transformer block (B=4, T=2048, C=1024, H=16, FF=4096) on 8
Trainium2 NeuronCores.

Sharding: sequence-parallel, zero collectives. Core c handles batch
b = c // 2 and query half r = c % 2. The host permutes each core's
tokens (order pi): own 256-token blocks {2s+r : s=0..3} first
(positions 0:1024), the remaining blocks after (1024:2048), so every
slice offset in the SPMD program is static and identical on all cores.

Attention runs in 4 query slots of 256 tokens. Slot s attends to kv
positions [0, 256(s+1)) (own side) and [1024, 1024+256(s+1)) (other
side) - uniform extents; causality inside the boundary chunk-pairs is
enforced with per-core bf16 mask data.

All GEMM operands are bf16 (fp32 PSUM accumulation). Scores matmuls
are row-packed: head-pair scores run as two concurrent 64x128 PE
tiles. A@V uses an appended ones-column in V (M=65) to produce softmax
denominators for free. LN1 statistics are computed from a token-major
copy of x on the Scalar engine (activation accumulate) and bounced
through DRAM to become per-token rows; LN gamma is applied via
outer-product broadcast matmuls, LN beta is folded into the following
matmul's bias on the host.
"""
import numpy as np
import ml_dtypes

B, T, C = 4, 2048, 1024
H, D, FF = 16, 64, 4096
NC = 8
NKC = C // 128     # 8 feature chunks
NFFC = FF // 128   # 32
OWN = 1024
EPS = 1e-5
ISD = 1.0 / np.sqrt(D)

_STATE = {}


def _build_program():
    import concourse.bacc as bacc
    import concourse.mybir as mybir
    from concourse.tile import TileContext

    F32R = mybir.dt.float32r
    F32 = mybir.dt.float32
    BF16 = mybir.dt.bfloat16
    AF = mybir.ActivationFunctionType
    OP = mybir.AluOpType

    nc = bacc.Bacc("TRN2", target_bir_lowering=False, debug=False,
                   num_devices=NC)

    xt_d = nc.dram_tensor("xt", [128, NKC, T], F32, kind="ExternalInput")
    xk_d = nc.dram_tensor("xk", [128, 16, C], F32, kind="ExternalInput")
    xq_d = nc.dram_tensor("xq", [128, NKC, OWN], BF16, kind="ExternalInput")
    wq_d = nc.dram_tensor("wq", [128, 8, NKC, 128], BF16, kind="ExternalInput")
    wk_d = nc.dram_tensor("wk", [128, 8, NKC, 128], BF16, kind="ExternalInput")
    wv_d = nc.dram_tensor("wv", [128, 2, NKC, 512], BF16, kind="ExternalInput")
    wp_d = nc.dram_tensor("wp", [128, 8, NKC, 128], BF16, kind="ExternalInput")
    wf1_d = nc.dram_tensor("wf1", [NFFC, 128, NKC, 128], BF16,
                           kind="ExternalInput")
    wf2_d = nc.dram_tensor("wf2", [NKC, 128, NFFC, 128], BF16,
                           kind="ExternalInput")
    g1_d = nc.dram_tensor("g1", [128, NKC], F32, kind="ExternalInput")
    g2_d = nc.dram_tensor("g2", [128, NKC], F32, kind="ExternalInput")
    qb_d = nc.dram_tensor("qb", [128, NKC], F32, kind="ExternalInput")
    kb_d = nc.dram_tensor("kb", [128, NKC], F32, kind="ExternalInput")
    vbr_d = nc.dram_tensor("vbr", [1, 2, 512], F32R, kind="ExternalInput")
    pb_d = nc.dram_tensor("pb", [128, NKC], F32, kind="ExternalInput")
    fb1_d = nc.dram_tensor("fb1", [128, NFFC], F32, kind="ExternalInput")
    fb2_d = nc.dram_tensor("fb2", [128, NKC], F32, kind="ExternalInput")
    # masks[kv_p, slot, side(own/other), chunk_in_pair, q]
    masks_d = nc.dram_tensor("masks", [128, 4, 2, 2, 256], BF16,
                             kind="ExternalInput")
    out_d = nc.dram_tensor("out", [128, NKC, OWN], F32, kind="ExternalOutput")

    def mm(ps, lhsT, rhs, start, stop):
        nc.tensor.matmul(ps, lhsT, rhs, start=start, stop=stop)

    with TileContext(nc, pool_alloc_mode="queue") as tc:
        consts_cm = tc.tile_pool(name="consts", bufs=1)
        consts = consts_cm.__enter__()
        dram_cm = tc.tile_pool(name="drp", bufs=1, space="DRAM")
        drp = dram_cm.__enter__()

        onesrow = consts.tile([1, 128], F32R)
        nc.vector.memset(onesrow.bitcast(F32), 1.0)
        eps128 = consts.tile([128, 1], F32)
        nc.vector.memset(eps128, EPS)
        eps1 = consts.tile([1, 1], F32)
        nc.vector.memset(eps1, EPS)
        g1t = consts.tile([128, NKC], F32)
        nc.sync.dma_start(out=g1t, in_=g1_d[:, :])
        g2t = consts.tile([128, NKC], F32)
        nc.sync.dma_start(out=g2t, in_=g2_d[:, :])
        qbt = consts.tile([128, NKC], F32)
        nc.sync.dma_start(out=qbt, in_=qb_d[:, :])
        kbt = consts.tile([128, NKC], F32)
        nc.sync.dma_start(out=kbt, in_=kb_d[:, :])
        pbt = consts.tile([128, NKC], F32)
        nc.sync.dma_start(out=pbt, in_=pb_d[:, :])
        fb1t = consts.tile([128, NFFC], F32)
        nc.sync.dma_start(out=fb1t, in_=fb1_d[:, :])
        fb2t = consts.tile([128, NKC], F32)
        nc.sync.dma_start(out=fb2t, in_=fb2_d[:, :])

        mu_d = drp.tile([16, 128], F32R)
        rs_d = drp.tile([16, 128], F32R)

        # ---- persistent activations (whole program) ----
        pers_cm = tc.tile_pool(name="pers", bufs=1)
        pers = pers_cm.__enter__()
        yt = pers.tile([128, NKC, OWN], BF16)
        vbb = pers.tile([128, 2, 512], BF16)

        # ---- K/Q/V outputs (freed after attention) ----
        qk_cm = tc.tile_pool(name="qkp", bufs=1)
        qkp = qk_cm.__enter__()
        kt = qkp.tile([128, NKC, T], BF16)
        qt = qkp.tile([128, NKC, OWN], BF16)
        vt = qkp.tile([128, 16, H, 65], BF16)
        nc.vector.memset(vt[:, :, :, 64:65], 1.0)
        lnx_cm = tc.tile_pool(name="lnxp", bufs=1)
        lnxp = lnx_cm.__enter__()
        lnx = lnxp.tile([128, NKC, T], BF16)

        # ======== Phase A: LN1 stats from token-major x ========
        wqk_cm = tc.tile_pool(name="wqkp", bufs=2)
        wqkp = wqk_cm.__enter__()
        rows_cm = tc.tile_pool(name="rows", bufs=4)
        rows = rows_cm.__enter__()
        xk_cm = tc.tile_pool(name="xkp", bufs=2)
        xkp = xk_cm.__enter__()
        st_cm = tc.tile_pool(name="stw", bufs=4)
        stw = st_cm.__enter__()
        scr_cm = tc.tile_pool(name="scrp", bufs=1)
        scrp = scr_cm.__enter__()
        scr1 = scrp.tile([128, C], BF16)
        scr2 = scrp.tile([128, C], BF16)
        murows, rsrows = [], []

        for blk in range(16):
            xkb = xkp.tile([128, C], F32, tag="xk")
            nc.sync.dma_start(out=xkb, in_=xk_d[:, blk, :])
            s_sum = stw.tile([128, 1], F32, tag="ssum")
            nc.scalar.activation(out=scr1, in_=xkb, func=AF.Copy,
                                 accum_out=s_sum)
            s_sq = stw.tile([128, 1], F32, tag="ssq")
            nc.scalar.activation(out=scr2, in_=xkb, func=AF.Square,
                                 accum_out=s_sq)
            mu = stw.tile([128, 1], F32R, tag="mu")
            with nc.allow_low_precision(reason="f32r mu"):
                nc.vector.tensor_scalar_mul(out=mu, in0=s_sum,
                                            scalar1=1.0 / C)
            mu2 = stw.tile([128, 1], F32, tag="mu2")
            nc.vector.tensor_mul(out=mu2, in0=mu.bitcast(F32),
                                 in1=mu.bitcast(F32))
            var = stw.tile([128, 1], F32, tag="var")
            nc.vector.scalar_tensor_tensor(
                out=var, in0=s_sq, scalar=1.0 / C, in1=mu2,
                op0=OP.mult, op1=OP.subtract)
            sd = stw.tile([128, 1], F32, tag="sd")
            nc.scalar.activation(out=sd, in_=var, func=AF.Sqrt,
                                 bias=eps128, scale=1.0)
            rstd = stw.tile([128, 1], F32R, tag="rstd")
            with nc.allow_low_precision(reason="f32r rstd"):
                nc.vector.reciprocal(out=rstd, in_=sd)
            nc.sync.dma_start(out=mu_d[blk:blk + 1, :], in_=mu)
            nc.sync.dma_start(out=rs_d[blk:blk + 1, :], in_=rstd)
            if blk % 4 == 3:
                g = blk // 4
                mrow = rows.tile([1, 512], F32R, tag="mu")
                nc.sync.dma_start(
                    out=mrow,
                    in_=mu_d[g * 4:(g + 1) * 4, :].rearrange("b p -> (b p)"))
                murows.append(mrow)
                rrow = rows.tile([1, 512], F32R, tag="rs")
                nc.sync.dma_start(
                    out=rrow,
                    in_=rs_d[g * 4:(g + 1) * 4, :].rearrange("b p -> (b p)"))
                rsrows.append(rrow)

        # ======== Phase B+C: LN1 apply + K/Q projections ========
        xto_cm = tc.tile_pool(name="xtop", bufs=2)
        xtop = xto_cm.__enter__()
        tw_cm = tc.tile_pool(name="twp", bufs=2)
        tw = tw_cm.__enter__()
        bc_cm = tc.tile_pool(name="bcps", bufs=2, space="PSUM")
        bcps = bc_cm.__enter__()
        qkps_cm = tc.tile_pool(name="qkps", bufs=4, space="PSUM")
        qkps = qkps_cm.__enter__()

        for tb in range(4):
            sl = slice(tb * 512, (tb + 1) * 512)
            xob = xtop.tile([128, NKC, 512], F32, tag="xo")
            nc.sync.dma_start(out=xob, in_=xt_d[:, :, sl])
            xsrc = xob[:, :, :]
            mu_ps = bcps.tile([128, 512], F32, tag="mub")
            mm(mu_ps, onesrow, murows[tb], True, True)
            rs_ps = bcps.tile([128, 512], F32, tag="rsb")
            mm(rs_ps, onesrow, rsrows[tb], True, True)
            for k in range(NKC):
                t1 = tw.tile([128, 512], F32, tag="t1")
                nc.vector.tensor_sub(out=t1, in0=xsrc[:, k, :], in1=mu_ps)
                with nc.allow_low_precision(reason="bf16 ln1 output"):
                    nc.vector.scalar_tensor_tensor(
                        out=lnx[:, k, sl], in0=t1, scalar=g1t[:, k:k + 1],
                        in1=rs_ps, op0=OP.mult, op1=OP.mult)
            # K for this token block
            for oc in range(8):
                wkt = wqkp.tile([128, NKC, 128], BF16, tag="wk")
                nc.sync.dma_start(out=wkt, in_=wk_d[:, oc, :, :])
                ps = qkps.tile([128, 512], F32, tag="mm")
                for k in range(NKC):
                    mm(ps, wkt[:, k, :], lnx[:, k, sl], k == 0,
                       k == NKC - 1)
                with nc.allow_low_precision(reason="bf16 k"):
                    nc.vector.tensor_scalar(
                        out=kt[:, oc, sl], in0=ps, scalar1=kbt[:, oc:oc + 1],
                        scalar2=0.0, op0=OP.add, op1=OP.add)
            # Q (own half only), scaled by 1/sqrt(D)
            if tb < 2:
                for oc in range(8):
                    wqt = wqkp.tile([128, NKC, 128], BF16, tag="wq")
                    nc.sync.dma_start(out=wqt, in_=wq_d[:, oc, :, :])
                    ps = qkps.tile([128, 512], F32, tag="mm")
                    for k in range(NKC):
                        mm(ps, wqt[:, k, :], lnx[:, k, sl], k == 0,
                           k == NKC - 1)
                    with nc.allow_low_precision(reason="bf16 q"):
                        nc.vector.tensor_scalar(
                            out=qt[:, oc, sl], in0=ps,
                            scalar1=qbt[:, oc:oc + 1], scalar2=ISD,
                            op0=OP.add, op1=OP.mult)

        qkps_cm.__exit__(None, None, None)
        bc_cm.__exit__(None, None, None)
        tw_cm.__exit__(None, None, None)
        xto_cm.__exit__(None, None, None)
        scr_cm.__exit__(None, None, None)
        st_cm.__exit__(None, None, None)
        xk_cm.__exit__(None, None, None)
        rows_cm.__exit__(None, None, None)
        wqk_cm.__exit__(None, None, None)

        # ======== Phase D: V projection (dense standalone) ========
        wv_cm = tc.tile_pool(name="wvp", bufs=1)
        wvp = wv_cm.__enter__()
        wvt = wvp.tile([128, 2, NKC, 512], BF16)
        nc.sync.dma_start(out=wvt, in_=wv_d[:, :, :, :])
        vbr = wvp.tile([1, 2, 512], F32R)
        nc.sync.dma_start(out=vbr, in_=vbr_d[:, :, :])
        vb_cm = tc.tile_pool(name="vbps", bufs=3, space="PSUM")
        vbps = vb_cm.__enter__()
        for g in range(2):
            ps = vbps.tile([128, 512], F32, tag="vmm")
            mm(ps, onesrow, vbr[0:1, g, :], True, True)
            with nc.allow_low_precision(reason="bf16 vb"):
                nc.vector.tensor_copy(out=vbb[:, g, :], in_=ps)
        for cch in range(16):
            for g in range(2):
                ps = vbps.tile([128, 512], F32, tag="vmm")
                csl = slice(cch * 128, (cch + 1) * 128)
                for k in range(NKC):
                    mm(ps, lnx[:, k, csl], wvt[:, g, k, :], k == 0,
                       k == NKC - 1)
                with nc.allow_low_precision(reason="bf16 v"):
                    nc.vector.scalar_tensor_tensor(
                        out=vt[:, cch, g * 8:(g + 1) * 8, 0:64],
                        in0=ps.rearrange("p (h d) -> p h d", h=8), scalar=1.0,
                        in1=vbb[:, g, :].rearrange("p (h d) -> p h d", h=8),
                        op0=OP.mult, op1=OP.add)
        vb_cm.__exit__(None, None, None)
        wv_cm.__exit__(None, None, None)
        lnx_cm.__exit__(None, None, None)

        # ======== Phase E: attention ========
        ptm_cm = tc.tile_pool(name="ptmp", bufs=5)
        ptmp = ptm_cm.__enter__()
        mt = ptmp.tile([128, 4, 2, 2, 256], BF16, tag="mt")
        nc.sync.dma_start(out=mt, in_=masks_d[:, :, :, :, :])
        ptm2_cm = tc.tile_pool(name="ptm2p", bufs=3)
        ptm2p = ptm2_cm.__enter__()
        nw_cm = tc.tile_pool(name="nwp", bufs=3)
        nwp = nw_cm.__enter__()
        ss_cm = tc.tile_pool(name="ssps", bufs=2, space="PSUM")
        ssps = ss_cm.__enter__()
        yy_cm = tc.tile_pool(name="yyps", bufs=2, space="PSUM")
        yyps = yy_cm.__enter__()
        nb_cm = tc.tile_pool(name="nbps", bufs=2, space="PSUM")
        nbps = nb_cm.__enter__()

        for s in range(4):
            qsl = slice(s * 256, (s + 1) * 256)
            # batches: (side, b) with kv chunk pair at side*8 + 2b, +1
            batches = [(0, b) for b in range(s + 1)] + \
                      [(1, b) for b in range(s + 1)]
            n_mm = len(batches) * 4
            for hp in range(8):
                ha, hb = 2 * hp, 2 * hp + 1

                def score_batch(side, b):
                    # layout [head, chunk, q]: the head index selects the
                    # PSUM bank so the two concurrent row-tiles never
                    # touch the same bank (HW hang otherwise)
                    c0 = side * 8 + 2 * b
                    ps_s = ssps.tile([128, 2, 2, 256], F32, tag="s")
                    for jc in range(2):
                        csl = slice((c0 + jc) * 128, (c0 + jc + 1) * 128)
                        mm(ps_s[:, 0, jc, :], kt[0:64, hp, csl],
                           qt[0:64, hp, qsl], True, True)
                        mm(ps_s[:, 1, jc, :], kt[64:128, hp, csl],
                           qt[64:128, hp, qsl], True, True)
                    ptm = ptmp.tile([128, 2, 2, 256], BF16, tag="pe")
                    with nc.allow_low_precision(reason="bf16 exp"):
                        nc.scalar.activation(out=ptm, in_=ps_s, func=AF.Exp)
                    if b == s:  # boundary pair: apply causal mask
                        ptmm = ptm2p.tile([128, 2, 2, 256], BF16, tag="pm")
                        with nc.allow_low_precision(reason="bf16 mask"):
                            for j in range(2):
                                nc.vector.tensor_mul(
                                    out=ptmm[:, j, :, :], in0=ptm[:, j, :, :],
                                    in1=mt[:, s, side, :, :])
                        return (c0, ptmm)
                    return (c0, ptm)

                ps_y = yyps.tile([65, 2, 256], F32, tag="y")
                i = 0
                prev = None
                # software pipeline: scores(b+1) emitted before A@V(b)
                for bi in range(len(batches) + 1):
                    cur = (score_batch(*batches[bi])
                           if bi < len(batches) else None)
                    if prev is not None:
                        c0, ptm = prev
                        for jc in range(2):
                            for (j, h) in ((0, ha), (1, hb)):
                                mm(ps_y[:, j, :], vt[:, c0 + jc, h, :],
                                   ptm[:, j, jc, :], i == 0, i == n_mm - 1)
                                i += 1
                    prev = cur
                # normalize: y / denom (denom = row 64)
                rdt = nwp.tile([1, 2, 256], F32R, tag="rd")
                ps_nb = nbps.tile([64, 2, 256], F32, tag="nb")
                for j in range(2):
                    with nc.allow_low_precision(reason="softmax denom"):
                        nc.vector.reciprocal(out=rdt[:, j, :],
                                             in_=ps_y[64:65, j, :])
                    mm(ps_nb[:, j, :], onesrow[:, 0:64],
                       rdt[:, j, :], True, True)
                rb = nwp.tile([64, 2, 256], F32, tag="rb")
                nc.vector.tensor_copy(out=rb, in_=ps_nb)
                with nc.allow_low_precision(reason="bf16 y"):
                    nc.vector.tensor_mul(out=yt[0:64, hp, qsl],
                                         in0=ps_y[0:64, 0, :],
                                         in1=rb[:, 0, :])
                    ytm = nwp.tile([64, 256], BF16, tag="ytm")
                    nc.vector.tensor_mul(out=ytm, in0=ps_y[0:64, 1, :],
                                         in1=rb[:, 1, :])
                nc.sync.dma_start(out=yt[64:128, hp, qsl], in_=ytm)

        nb_cm.__exit__(None, None, None)
        yy_cm.__exit__(None, None, None)
        ss_cm.__exit__(None, None, None)
        nw_cm.__exit__(None, None, None)
        ptm2_cm.__exit__(None, None, None)
        ptm_cm.__exit__(None, None, None)
        qk_cm.__exit__(None, None, None)

        # ======== Phase F/G/H: proj -> LN2 -> MLP (interleaved) ========
        x2_cm = tc.tile_pool(name="x2p", bufs=1)
        x2p = x2_cm.__enter__()
        x2t = x2p.tile([128, NKC, OWN], BF16)
        prw_cm = tc.tile_pool(name="prw", bufs=2)
        prw = prw_cm.__enter__()
        xtq_cm = tc.tile_pool(name="xtqp", bufs=1)
        xtqp = xtq_cm.__enter__()
        xtq = xtqp.tile([128, NKC, OWN], BF16)      # own-half x (residual)
        for oc in range(8):
            nc.sync.dma_start(out=xtq[:, oc, :], in_=xq_d[:, oc, :])
        tp1_cm = tc.tile_pool(name="tps1", bufs=2, space="PSUM")
        tp1 = tp1_cm.__enter__()
        tp2_cm = tc.tile_pool(name="tps2", bufs=1, space="PSUM")
        tp2 = tp2_cm.__enter__()
        tp3_cm = tc.tile_pool(name="tps3", bufs=2, space="PSUM")
        tp3 = tp3_cm.__enter__()
        ln2_cm = tc.tile_pool(name="ln2p", bufs=1)
        ln2p = ln2_cm.__enter__()
        ln2x = ln2p.tile([128, NKC, OWN], BF16)
        ones128b = ln2p.tile([128, 1], BF16)
        nc.vector.memset(ones128b, 1.0)
        l2w_cm = tc.tile_pool(name="l2w", bufs=3)
        l2w = l2w_cm.__enter__()

        def proj(tb):
            sl = slice(tb * 512, (tb + 1) * 512)
            for oc in range(8):
                wpt = prw.tile([128, NKC, 128], BF16, tag="wp")
                nc.sync.dma_start(out=wpt, in_=wp_d[:, oc, :, :])
                ps = tp1.tile([128, 512], F32, tag="pmm")
                for k in range(NKC):
                    mm(ps, wpt[:, k, :], yt[:, k, sl], k == 0,
                       k == NKC - 1)
                with nc.allow_low_precision(reason="bf16 x2"):
                    nc.vector.scalar_tensor_tensor(
                        out=x2t[:, oc, sl], in0=ps, scalar=pbt[:, oc:oc + 1],
                        in1=xtq[:, oc, sl], op0=OP.add, op1=OP.add)

        def ln2_stats_sum(tb):
            sl = slice(tb * 512, (tb + 1) * 512)
            ps_s = tp1.tile([1, 512], F32, tag="l2s")
            for k in range(NKC):
                mm(ps_s, ones128b, x2t[:, k, sl], k == 0, k == NKC - 1)
            return ps_s

        def ln2_stats_sq(tb):
            sl = slice(tb * 512, (tb + 1) * 512)
            sqs = []
            for k in range(NKC):
                sq = l2w.tile([128, 512], BF16, tag="sq")
                nc.scalar.activation(out=sq, in_=x2t[:, k, sl],
                                     func=AF.Square)
                sqs.append(sq)
            ps_q = tp1.tile([1, 512], F32, tag="l2s")
            for k in range(NKC):
                mm(ps_q, ones128b, sqs[k], k == 0, k == NKC - 1)
            return ps_q

        def ln2_rows_bcast(ps_s, ps_q):
            mu = l2w.tile([1, 512], F32R, tag="mu")
            with nc.allow_low_precision(reason="f32r mu2"):
                nc.vector.tensor_scalar_mul(out=mu, in0=ps_s,
                                            scalar1=1.0 / C)
            mu2 = l2w.tile([1, 512], F32, tag="mu2")
            nc.vector.tensor_mul(out=mu2, in0=mu.bitcast(F32),
                                 in1=mu.bitcast(F32))
            var = l2w.tile([1, 512], F32, tag="var")
            nc.vector.scalar_tensor_tensor(
                out=var, in0=ps_q, scalar=1.0 / C, in1=mu2,
                op0=OP.mult, op1=OP.subtract)
            sd = l2w.tile([1, 512], F32, tag="sd")
            nc.scalar.activation(out=sd, in_=var, func=AF.Sqrt,
                                 bias=eps1, scale=1.0)
            rstd = l2w.tile([1, 512], F32R, tag="rstd")
            with nc.allow_low_precision(reason="f32r rstd2"):
                nc.vector.reciprocal(out=rstd, in_=sd)
            mu_ps = tp2.tile([128, 512], F32, tag="mub")
            mm(mu_ps, onesrow, mu, True, True)
            rs_ps = tp2.tile([128, 512], F32, tag="rsb")
            mm(rs_ps, onesrow, rstd, True, True)
            return mu_ps, rs_ps

        def ln2_apply(tb, mu_ps, rs_ps):
            sl = slice(tb * 512, (tb + 1) * 512)
            for k in range(NKC):
                t1 = l2w.tile([128, 512], F32, tag="t1")
                nc.vector.tensor_sub(out=t1, in0=x2t[:, k, sl], in1=mu_ps)
                with nc.allow_low_precision(reason="bf16 ln2"):
                    nc.vector.scalar_tensor_tensor(
                        out=ln2x[:, k, sl], in0=t1, scalar=g2t[:, k:k + 1],
                        in1=rs_ps, op0=OP.mult, op1=OP.mult)

        proj(0)
        s0 = ln2_stats_sum(0)
        proj(1)
        q0 = ln2_stats_sq(0)
        mu0, rs0 = ln2_rows_bcast(s0, q0)
        s1 = ln2_stats_sum(1)
        ln2_apply(0, mu0, rs0)
        q1 = ln2_stats_sq(1)
        mu1, rs1 = ln2_rows_bcast(s1, q1)
        ln2_apply(1, mu1, rs1)

        prw_exit_late = prw_cm  # freed at the end (LIFO)

        # ---- MLP: weight-resident loops over both token halves ----
        mw1_cm = tc.tile_pool(name="mw1", bufs=3)
        mw1 = mw1_cm.__enter__()
        mw2_cm = tc.tile_pool(name="mw2", bufs=2)
        mw2 = mw2_cm.__enter__()
        m1_cm = tc.tile_pool(name="m1p", bufs=1)
        m1p = m1_cm.__enter__()
        m1ta = m1p.tile([128, NFFC, 512], BF16, tag="m1a")
        m1tb = m1p.tile([128, NFFC, 512], BF16, tag="m1b")
        m1ts = [m1ta, m1tb]
        mo_cm = tc.tile_pool(name="mo", bufs=3)
        mo = mo_cm.__enter__()

        for ffc in range(NFFC):
            wt = mw1.tile([128, NKC, 128], BF16, tag="w1")
            nc.sync.dma_start(out=wt, in_=wf1_d[ffc])
            for tb in range(2):
                sl = slice(tb * 512, (tb + 1) * 512)
                ps = tp3.tile([128, 512], F32, tag="mmm")
                for k in range(NKC):
                    mm(ps, wt[:, k, :], ln2x[:, k, sl], k == 0, k == NKC - 1)
                with nc.allow_low_precision(reason="bf16 m1"):
                    nc.vector.tensor_scalar(
                        out=m1ts[tb][:, ffc, :], in0=ps,
                        scalar1=fb1t[:, ffc:ffc + 1], scalar2=0.0,
                        op0=OP.add, op1=OP.max)
        for oc in range(NKC):
            wt2 = mw2.tile([128, NFFC, 128], BF16, tag="w2")
            nc.sync.dma_start(out=wt2, in_=wf2_d[oc])
            for tb in range(2):
                sl = slice(tb * 512, (tb + 1) * 512)
                ps = tp3.tile([128, 512], F32, tag="mmm")
                for k in range(NFFC):
                    mm(ps, wt2[:, k, :], m1ts[tb][:, k, :], k == 0,
                       k == NFFC - 1)
                ot = mo.tile([128, 512], F32, tag="ot")
                nc.vector.scalar_tensor_tensor(
                    out=ot, in0=ps, scalar=fb2t[:, oc:oc + 1],
                    in1=x2t[:, oc, sl], op0=OP.add, op1=OP.add)
                nc.sync.dma_start(out=out_d[:, oc, sl], in_=ot)

        mo_cm.__exit__(None, None, None)
        m1_cm.__exit__(None, None, None)
        mw2_cm.__exit__(None, None, None)
        mw1_cm.__exit__(None, None, None)
        l2w_cm.__exit__(None, None, None)
        ln2_cm.__exit__(None, None, None)
        tp3_cm.__exit__(None, None, None)
        tp2_cm.__exit__(None, None, None)
        tp1_cm.__exit__(None, None, None)
        xtq_cm.__exit__(None, None, None)
        prw_cm.__exit__(None, None, None)
        x2_cm.__exit__(None, None, None)
        pers_cm.__exit__(None, None, None)
        dram_cm.__exit__(None, None, None)
        consts_cm.__exit__(None, None, None)

    nc.compile()
    return nc


class _SpmdRunner:
    def __init__(self, nc, n_cores=NC):
        import jax
        from jax.sharding import Mesh, PartitionSpec
        from jax.experimental.shard_map import shard_map
        import concourse.mybir as mybir
        from concourse import bass2jax
        bass2jax.install_neuronx_cc_hook()
        self.jax = jax
        self.n_cores = n_cores
        partition_name = (
            nc.partition_id_tensor.name if nc.partition_id_tensor else None)
        in_names, out_names, out_avals = [], [], []
        for alloc in nc.m.functions[0].allocations:
            if not isinstance(alloc, mybir.MemoryLocationSet):
                continue
            name = alloc.memorylocations[0].name
            if alloc.kind == "ExternalInput":
                if name != partition_name:
                    in_names.append(name)
            elif alloc.kind == "ExternalOutput":
                out_names.append(name)
                out_avals.append(jax.core.ShapedArray(
                    tuple(alloc.tensor_shape), mybir.dt.np(alloc.dtype)))
        self.in_names = in_names
        self.out_names = out_names
        self.out_avals = out_avals
        all_in = in_names + out_names
        if partition_name is not None:
            all_in.append(partition_name)

        def _body(*args):
            operands = list(args)
            if partition_name is not None:
                operands.append(bass2jax.partition_id_tensor())
            outs = bass2jax._bass_exec_p.bind(
                *operands, out_avals=tuple(out_avals),
                in_names=tuple(all_in), out_names=tuple(out_names),
                lowering_input_output_aliases=(),
                sim_require_finite=True, sim_require_nnan=True, nc=nc)
            return tuple(outs)

        devices = jax.devices()[:n_cores]
        self.mesh = Mesh(np.asarray(devices), ("core",))
        n_io = len(in_names) + len(out_names)
        self.fn = jax.jit(
            shard_map(_body, mesh=self.mesh,
                      in_specs=(PartitionSpec("core"),) * n_io,
                      out_specs=(PartitionSpec("core"),) * len(out_names),
                      check_rep=False),
            keep_unused=True)
        self._dev_in = None

    def put_inputs(self, in_maps):
        from jax.sharding import NamedSharding, PartitionSpec
        jax = self.jax
        sh = NamedSharding(self.mesh, PartitionSpec("core"))
        concat = []
        for name in self.in_names:
            arrs = [np.asarray(in_maps[c][name]) for c in range(self.n_cores)]
            concat.append(jax.device_put(np.concatenate(arrs, axis=0), sh))
        for av in self.out_avals:
            z = np.zeros((self.n_cores * av.shape[0], *av.shape[1:]), av.dtype)
            concat.append(jax.device_put(z, sh))
        self._dev_in = concat

    def run(self):
        jax = self.jax
        outs = self.fn(*self._dev_in)
        jax.block_until_ready(outs)
        results = []
        for c in range(self.n_cores):
            d = {}
            for i, name in enumerate(self.out_names):
                av = self.out_avals[i]
                d[name] = np.asarray(outs[i]).reshape(
                    self.n_cores, *av.shape)[c]
            results.append(d)
        return results

    def time_exec(self, warmup=3, m1=4, m2=12, reps=3, trials=6):
        """Estimate per-call device time by dispatching bursts of m1 and
        m2 back-to-back calls and differencing, which cancels the
        constant dispatch/RTT overhead of the axon tunnel."""
        import time
        jax = self.jax
        for _ in range(warmup):
            jax.block_until_ready(self.fn(*self._dev_in))

        def burst(m):
            t0 = time.perf_counter()
            outs = None
            for _ in range(m):
                outs = self.fn(*self._dev_in)
            jax.block_until_ready(outs)
            return time.perf_counter() - t0

        t1s, t2s = [], []
        for _ in range(trials):
            for _ in range(reps):
                t1s.append(burst(m1))
                t2s.append(burst(m2))
        return (min(t2s) - min(t1s)) / (m2 - m1)


def _get_runner():
    if "runner" not in _STATE:
        nc = _build_program()
        _STATE["runner"] = _SpmdRunner(nc)
    return _STATE["runner"]


def _perm(r):
    """Token permutation for core half r: own 256-blocks {2s+r} first."""
    own = np.concatenate(
        [np.arange(256 * (2 * s + r), 256 * (2 * s + r) + 256)
         for s in range(4)])
    oth = np.concatenate(
        [np.arange(256 * (2 * s + 1 - r), 256 * (2 * s + 1 - r) + 256)
         for s in range(4)])
    return np.concatenate([own, oth])


def _prep_in_maps(x, W_attn, W_proj, b_proj, W_fc1, b_fc1, W_fc2, b_fc2,
                  ln1_g, ln1_b, ln2_g, ln2_b):
    f32 = np.float32
    bf16 = ml_dtypes.bfloat16
    x = np.asarray(x, f32)
    W_attn = np.asarray(W_attn, f32)
    Wq, Wk, Wv = W_attn[:, 0:C], W_attn[:, C:2 * C], W_attn[:, 2 * C:3 * C]
    W_proj = np.asarray(W_proj, f32)
    W_fc1 = np.asarray(W_fc1, f32)
    W_fc2 = np.asarray(W_fc2, f32)
    ln1_b = np.asarray(ln1_b, f32)
    ln2_b = np.asarray(ln2_b, f32)

    def lhs_tiles(W, nout):
        # [C, nout*128] -> [128p, nout, NKC, 128m]
        return np.ascontiguousarray(
            W.reshape(NKC, 128, nout, 128).transpose(1, 2, 0, 3)).astype(bf16)

    wq = lhs_tiles(Wq, 8)
    wk = lhs_tiles(Wk, 8)
    wv = np.ascontiguousarray(
        Wv.reshape(NKC, 128, 2, 512).transpose(1, 2, 0, 3)).astype(bf16)
    wp = lhs_tiles(W_proj, 8)
    wf1 = np.ascontiguousarray(
        W_fc1.reshape(NKC, 128, NFFC, 128).transpose(2, 1, 0, 3)).astype(bf16)
    wf2 = np.ascontiguousarray(
        W_fc2.reshape(NFFC, 128, NKC, 128).transpose(2, 1, 0, 3)).astype(bf16)

    def vec(v, nk):
        return np.ascontiguousarray(np.asarray(v, f32).reshape(nk, 128).T)

    qb = vec(ln1_b @ Wq, NKC)
    kb = vec(ln1_b @ Wk, NKC)
    vbr = np.ascontiguousarray((ln1_b @ Wv).reshape(1, 2, 512))
    fb1 = vec(np.asarray(b_fc1, f32) + ln2_b @ W_fc1, NFFC)

    shared = {
        "wq": wq, "wk": wk, "wv": wv, "wp": wp, "wf1": wf1, "wf2": wf2,
        "g1": vec(ln1_g, NKC), "g2": vec(ln2_g, NKC),
        "qb": qb, "kb": kb, "vbr": vbr,
        "pb": vec(b_proj, NKC), "fb1": fb1, "fb2": vec(b_fc2, NKC),
    }

    in_maps = []
    for c in range(NC):
        b, r = c // 2, c % 2
        pi = _perm(r)
        xs = x[b][pi]                          # [T, C] pi-ordered
        xt = np.ascontiguousarray(
            xs.T.reshape(NKC, 128, T).transpose(1, 0, 2))   # [128, NKC, T]
        xk = np.ascontiguousarray(
            xs.reshape(16, 128, C).transpose(1, 0, 2))      # [128, 16, C]
        xq = np.ascontiguousarray(
            xs[0:OWN].T.reshape(NKC, 128, OWN).transpose(1, 0, 2)).astype(bf16)
        # masks[kv_p, slot, side, chunk_in_pair, q]
        m = np.zeros((128, 4, 2, 2, 256), np.float32)
        for s in range(4):
            gq = 256 * (2 * s + r) + np.arange(256)
            for side, blk in ((0, 2 * s + r), (1, 2 * s + 1 - r)):
                kv = 256 * blk + np.arange(256)
                mm_ = (kv[:, None] <= gq[None, :]).astype(np.float32)
                m[:, s, side, 0, :] = mm_[0:128]
                m[:, s, side, 1, :] = mm_[128:256]
        d = {"xt": xt, "xk": xk, "xq": xq, "masks": m.astype(bf16)}
        d.update(shared)
        in_maps.append(d)
    return in_maps


def kernel(x, W_attn, W_proj, b_proj, W_fc1, b_fc1, W_fc2, b_fc2,
           ln1_g, ln1_b, ln2_g, ln2_b):
    runner = _get_runner()
    in_maps = _prep_in_maps(x, W_attn, W_proj, b_proj, W_fc1, b_fc1,
                            W_fc2, b_fc2, ln1_g, ln1_b, ln2_g, ln2_b)
    runner.put_inputs(in_maps)
    results = runner.run()
    out = np.empty((B, T, C), np.float32)
    for c in range(NC):
        b, r = c // 2, c % 2
        pi = _perm(r)
        ot = results[c]["out"]                # [128, NKC, OWN]
        feat = ot.transpose(1, 0, 2).reshape(C, OWN)
        out[b, pi[:OWN], :] = feat.T
    return out
